# revision 4
# baseline (speedup 1.0000x reference)
"""MoE FFN (flylora + ERA) Trainium2 kernel.

Expert-parallel over 8 NeuronCores: core e holds expert e's weights and
processes the tokens routed to it (top-2 routing computed on host as part of
the sharding step). All heavy math — the three [*,1024]x[1024,2048]-class
matmuls per expert plus the LoRA chain and ERA activation — runs on device.

Device-side layout trick: everything is computed transposed ([feature, token]),
so every matmul's stationary operand comes straight from the natural weight
layout and no on-device transposes are needed:
    hT[I,C]  = basW[H,I] as lhsT tiles, xeT[H,C] as moving     (mm1, both branches)
    loT[R,C] = downW[H,R] as lhsT, xeT moving                   (lora down)
    hT      += SCALING*upW[R,I] as lhsT, loT moving             (lora up, same psum)
    yT[H,C]  = downProj[I,H] as lhsT, prodT[I,C] moving         (mm2)
Biases fold to one per-partition vector: cb = base_b + S*(up_b + down_b@up_w).
ERA(x) = gelu_tanh(x) + 0.1*softplus(x); softplus = Ln(1+Exp(x)) because the
deployed ACT tables have no Softplus entry. Exp/Ln share one table set and
Gelu_apprx_tanh lives in another, so activations are batched by set.
"""

import sys

sys.path.insert(0, "/opt/trn_rl_repo")

import numpy as np
import ml_dtypes

import concourse.bass as bass  # noqa: F401
import concourse.mybir as mybir
import concourse.tile as tile
from concourse import bacc
from concourse.bass_utils import run_bass_kernel_spmd

H = 1024
I = 2048
E = 8
TOP_K = 2
R = 16
SCALING = 32.0 / 16.0
GAMMA = 0.1
KH = H // 128   # 8  k-tiles for H contraction
MI = I // 128   # 16 m-tiles over I
MH = H // 128   # 8  m-tiles over H (mm2 output)
KI = I // 128   # 16 k-tiles for I contraction
GRP = 4         # m-tiles per mm1 weight DMA slab
DGRP = 2        # m-tiles per mm2 weight DMA slab
QMI = 4         # m-tiles per batched activation quarter

AF = mybir.ActivationFunctionType
DT = mybir.dt

_cache: dict = {}
_last_results = None  # BassKernelResults of the most recent run (for profiling)


def _build(C: int, nch: int, cw: int):
    nc = bacc.Bacc("TRN2", target_bir_lowering=False, debug=False, num_devices=8)

    d_xe = nc.dram_tensor("xe", [128, KH, C], DT.bfloat16, kind="ExternalInput").ap()
    d_gw = nc.dram_tensor("gw", [MI // GRP, 128, GRP * H], DT.bfloat16, kind="ExternalInput").ap()
    d_uw = nc.dram_tensor("uw", [MI // GRP, 128, GRP * H], DT.bfloat16, kind="ExternalInput").ap()
    d_dw = nc.dram_tensor("dw", [MH // DGRP, 128, DGRP * I], DT.bfloat16, kind="ExternalInput").ap()
    d_gdw = nc.dram_tensor("gdw", [128, KH * R], DT.bfloat16, kind="ExternalInput").ap()
    d_udw = nc.dram_tensor("udw", [128, KH * R], DT.bfloat16, kind="ExternalInput").ap()
    d_guw = nc.dram_tensor("guw", [R, I], DT.bfloat16, kind="ExternalInput").ap()
    d_uuw = nc.dram_tensor("uuw", [R, I], DT.bfloat16, kind="ExternalInput").ap()
    d_cbg = nc.dram_tensor("cbg", [128, MI], DT.float32, kind="ExternalInput").ap()
    d_cbu = nc.dram_tensor("cbu", [128, MI], DT.float32, kind="ExternalInput").ap()
    d_dbb = nc.dram_tensor("dbb", [128, MH], DT.float32, kind="ExternalInput").ap()
    d_y = nc.dram_tensor("y", [MH, 128, C], DT.float32, kind="ExternalOutput").ap()

    from contextlib import ExitStack

    with tile.TileContext(nc) as tc, ExitStack() as ctx:
        consts = ctx.enter_context(tc.tile_pool(name="consts", bufs=1))
        big = ctx.enter_context(tc.tile_pool(name="big", bufs=1))
        wpool = ctx.enter_context(tc.tile_pool(name="wpool", bufs=2))
        dpool = ctx.enter_context(tc.tile_pool(name="dpool", bufs=2))
        ypool = ctx.enter_context(tc.tile_pool(name="ypool", bufs=3))
        psum_bufs = 2 if nch >= 2 else 3
        ps1 = ctx.enter_context(tc.tile_pool(name="ps1", bufs=psum_bufs, space="PSUM"))
        ps2 = ctx.enter_context(tc.tile_pool(name="ps2", bufs=psum_bufs, space="PSUM"))

        if True:
            # --- constant / input loads ---
            xe = consts.tile([128, KH, C], DT.bfloat16)
            nc.sync.dma_start(out=xe, in_=d_xe)
            gdw = consts.tile([128, KH * R], DT.bfloat16)
            nc.sync.dma_start(out=gdw, in_=d_gdw)
            udw = consts.tile([128, KH * R], DT.bfloat16)
            nc.sync.dma_start(out=udw, in_=d_udw)
            guw = consts.tile([R, I], DT.bfloat16)
            nc.sync.dma_start(out=guw, in_=d_guw)
            uuw = consts.tile([R, I], DT.bfloat16)
            nc.sync.dma_start(out=uuw, in_=d_uuw)
            cbg = consts.tile([128, MI], DT.float32)
            nc.sync.dma_start(out=cbg, in_=d_cbg)
            cbu = consts.tile([128, MI], DT.float32)
            nc.sync.dma_start(out=cbu, in_=d_cbu)
            dbb = consts.tile([128, MH], DT.float32)
            nc.sync.dma_start(out=dbb, in_=d_dbb)

            # --- lora down: loT[R, C] = down_w.T @ x.T, accumulated over KH ---
            lo_g = consts.tile([R, C], DT.bfloat16)
            lo_u = consts.tile([R, C], DT.bfloat16)
            for dwn, lo in ((gdw, lo_g), (udw, lo_u)):
                pl = ps2.tile([R, nch, 512], DT.float32, tag="acc2")
                for j in range(nch):
                    for k in range(KH):
                        nc.tensor.matmul(
                            pl[:, j, :cw],
                            dwn[:, k * R:(k + 1) * R],
                            xe[:, k, j * cw:(j + 1) * cw],
                            start=(k == 0),
                            stop=(k == KH - 1),
                        )
                for j in range(nch):
                    nc.vector.tensor_copy(lo[:, j * cw:(j + 1) * cw], pl[:, j, :cw])

            # --- big working buffers ---
            A = big.tile([128, MI, C], DT.float32)    # gate pre-act, then gelu(pre) in place
            Bb = big.tile([128, MI, C], DT.float32)   # exp(pre), then softplus in place
            U = big.tile([128, MI, C], DT.bfloat16)   # up branch (bias applied)
            GB = big.tile([128, MI, C], DT.bfloat16)  # ERA(gate)
            P = big.tile([128, MI, C], DT.bfloat16)   # gate*up, mm2 moving operand

            def mm1_branch(d_w, upw, lo, cb, dst, dst_dtype_bias, do_expln):
                for g in range(MI // GRP):
                    wslab = wpool.tile([128, GRP * H], DT.bfloat16)
                    nc.sync.dma_start(out=wslab, in_=d_w[g])
                    for mi in range(GRP):
                        m = g * GRP + mi
                        pg = ps1.tile([128, nch, 512], DT.float32)
                        for j in range(nch):
                            for k in range(KH):
                                nc.tensor.matmul(
                                    pg[:, j, :cw],
                                    wslab[:, mi * H + k * 128: mi * H + (k + 1) * 128],
                                    xe[:, k, j * cw:(j + 1) * cw],
                                    start=(k == 0),
                                    stop=False,
                                )
                            nc.tensor.matmul(
                                pg[:, j, :cw],
                                upw[:, m * 128:(m + 1) * 128],
                                lo[:, j * cw:(j + 1) * cw],
                                start=False,
                                stop=True,
                            )
                        nc.scalar.activation(
                            dst[:, m, :].rearrange("p (j c) -> p j c", j=nch),
                            pg[:, :, :cw],
                            AF.Identity,
                            bias=cb[:, m:m + 1],
                        )
                    if do_expln and (g + 1) * GRP % QMI == 0:
                        q0 = (g + 1) * GRP - QMI
                        a_q = A[:, q0:q0 + QMI, :].rearrange("p m c -> p (m c)")
                        b_q = Bb[:, q0:q0 + QMI, :].rearrange("p m c -> p (m c)")
                        nc.scalar.activation(b_q, a_q, AF.Exp)
                        nc.scalar.activation(b_q, b_q, AF.Ln, bias=1.0)

            # gate branch (evac to A in f32; exp/ln quarters interleaved)
            mm1_branch(d_gw, guw, lo_g, cbg, A, DT.float32, True)

            # up branch; after each quarter of up evacs, finish that quarter of
            # the gate path (gelu + combine) and the product
            for g in range(MI // GRP):
                wslab = wpool.tile([128, GRP * H], DT.bfloat16)
                nc.sync.dma_start(out=wslab, in_=d_uw[g])
                for mi in range(GRP):
                    m = g * GRP + mi
                    pg = ps1.tile([128, nch, 512], DT.float32)
                    for j in range(nch):
                        for k in range(KH):
                            nc.tensor.matmul(
                                pg[:, j, :cw],
                                wslab[:, mi * H + k * 128: mi * H + (k + 1) * 128],
                                xe[:, k, j * cw:(j + 1) * cw],
                                start=(k == 0),
                                stop=False,
                            )
                        nc.tensor.matmul(
                            pg[:, j, :cw],
                            uuw[:, m * 128:(m + 1) * 128],
                            lo_u[:, j * cw:(j + 1) * cw],
                            start=False,
                            stop=True,
                        )
                    nc.scalar.activation(
                        U[:, m, :].rearrange("p (j c) -> p j c", j=nch),
                        pg[:, :, :cw],
                        AF.Identity,
                        bias=cbu[:, m:m + 1],
                    )
                if (g + 1) * GRP % QMI == 0:
                    q0 = (g + 1) * GRP - QMI
                    a_q = A[:, q0:q0 + QMI, :].rearrange("p m c -> p (m c)")
                    b_q = Bb[:, q0:q0 + QMI, :].rearrange("p m c -> p (m c)")
                    u_q = U[:, q0:q0 + QMI, :].rearrange("p m c -> p (m c)")
                    gb_q = GB[:, q0:q0 + QMI, :].rearrange("p m c -> p (m c)")
                    p_q = P[:, q0:q0 + QMI, :].rearrange("p m c -> p (m c)")
                    nc.scalar.activation(a_q, a_q, AF.Gelu_apprx_tanh)
                    nc.vector.scalar_tensor_tensor(
                        gb_q, b_q, GAMMA, a_q,
                        mybir.AluOpType.mult, mybir.AluOpType.add,
                    )
                    nc.vector.tensor_mul(p_q, gb_q, u_q)

            # --- mm2: yT[H, C] = down_w.T @ prodT, accumulated over KI ---
            for g in range(MH // DGRP):
                dslab = dpool.tile([128, DGRP * I], DT.bfloat16)
                nc.sync.dma_start(out=dslab, in_=d_dw[g])
                for mi in range(DGRP):
                    m = g * DGRP + mi
                    py = ps2.tile([128, nch, 512], DT.float32, tag="acc2")
                    for j in range(nch):
                        for k in range(KI):
                            nc.tensor.matmul(
                                py[:, j, :cw],
                                dslab[:, mi * I + k * 128: mi * I + (k + 1) * 128],
                                P[:, k, j * cw:(j + 1) * cw],
                                start=(k == 0),
                                stop=(k == KI - 1),
                            )
                    yt = ypool.tile([128, C], DT.float32)
                    nc.scalar.activation(
                        yt.rearrange("p (j c) -> p j c", j=nch),
                        py[:, :, :cw],
                        AF.Identity,
                        bias=dbb[:, m:m + 1],
                    )
                    nc.sync.dma_start(out=d_y[m], in_=yt)

    nc.compile()
    return nc


def _pack_inputs(e, xf_b16, toks, counts, C, w):
    """Per-core input map for expert e; token block already chosen."""
    n = len(toks)
    xe = np.zeros((H, C), dtype=ml_dtypes.bfloat16)
    if n:
        xe[:, :n] = xf_b16[toks].T
    xe = np.ascontiguousarray(
        xe.reshape(KH, 128, C).transpose(1, 0, 2))  # [128, KH, C]

    def pack_mm1(wt):  # [H, I] -> [MI//GRP, 128, GRP*H]
        t = wt.reshape(KH, 128, MI, 128).transpose(2, 1, 0, 3).reshape(MI, 128, H)
        return np.ascontiguousarray(
            t.reshape(MI // GRP, GRP, 128, H).transpose(0, 2, 1, 3)
            .reshape(MI // GRP, 128, GRP * H))

    def pack_mm2(wt):  # [I, H] -> [MH//DGRP, 128, DGRP*I]
        t = wt.reshape(KI, 128, MH, 128).transpose(2, 1, 0, 3).reshape(MH, 128, I)
        return np.ascontiguousarray(
            t.reshape(MH // DGRP, DGRP, 128, I).transpose(0, 2, 1, 3)
            .reshape(MH // DGRP, 128, DGRP * I))

    b16 = ml_dtypes.bfloat16
    gw = pack_mm1(w["gate_base_w"][e].astype(b16))
    uw = pack_mm1(w["up_base_w"][e].astype(b16))
    dw = pack_mm2(w["down_w"][e].astype(b16))
    gdw = np.ascontiguousarray(
        w["gate_down_w"][e].astype(b16).reshape(KH, 128, R).transpose(1, 0, 2)
        .reshape(128, KH * R))
    udw = np.ascontiguousarray(
        w["up_down_w"][e].astype(b16).reshape(KH, 128, R).transpose(1, 0, 2)
        .reshape(128, KH * R))
    guw = (SCALING * w["gate_up_w"][e]).astype(b16)
    uuw = (SCALING * w["up_up_w"][e]).astype(b16)
    cbg = (w["gate_base_b"][e].astype(np.float64)
           + SCALING * (w["gate_up_b"][e].astype(np.float64)
                        + w["gate_down_b"][e].astype(np.float64)
                        @ w["gate_up_w"][e].astype(np.float64))).astype(np.float32)
    cbu = (w["up_base_b"][e].astype(np.float64)
           + SCALING * (w["up_up_b"][e].astype(np.float64)
                        + w["up_down_b"][e].astype(np.float64)
                        @ w["up_up_w"][e].astype(np.float64))).astype(np.float32)
    return {
        "xe": xe, "gw": gw, "uw": uw, "dw": dw, "gdw": gdw, "udw": udw,
        "guw": guw, "uuw": uuw,
        "cbg": np.ascontiguousarray(cbg.reshape(MI, 128).T),
        "cbu": np.ascontiguousarray(cbu.reshape(MI, 128).T),
        "dbb": np.ascontiguousarray(
            w["down_b"][e].astype(np.float32).reshape(MH, 128).T),
    }


def kernel(**inputs):
    global _last_results
    w = {k: np.asarray(v) for k, v in inputs.items()}
    x = w["x"]
    b, s, _ = x.shape
    T = b * s
    xf = x.reshape(T, H).astype(np.float32)

    # --- router (host; this determines the sharding) ---
    logits = xf @ w["router_w"].astype(np.float32) + w["router_b"].astype(np.float32)
    mx = logits.max(-1, keepdims=True)
    ex = np.exp(logits - mx)
    probs = ex / ex.sum(-1, keepdims=True)
    ti = np.argsort(-probs, axis=-1, kind="stable")[:, :TOP_K]
    tp = np.take_along_axis(probs, ti, axis=-1)
    tw = tp / tp.sum(-1, keepdims=True)

    p_mean = probs.mean(axis=0)
    f = np.bincount(ti.ravel(), minlength=E).astype(np.float32) / (T * TOP_K)
    aux_loss = np.float32(E * np.sum(f * p_mean))

    toks_all, wts_all = [], []
    for e in range(E):
        t_idx, slot = np.nonzero(ti == e)
        toks_all.append(t_idx)
        wts_all.append(tw[t_idx, slot].astype(np.float32))
    counts = np.array([len(t) for t in toks_all])

    xf_b16 = xf.astype(ml_dtypes.bfloat16)
    out_f = np.zeros((T, H), dtype=np.float32)

    # token blocks of at most 1024 per expert per kernel launch
    CB = 1024
    n_blocks = max(1, int(-(-counts.max() // CB)))
    for blk in range(n_blocks):
        blk_toks = [t[blk * CB:(blk + 1) * CB] for t in toks_all]
        blk_max = max(len(t) for t in blk_toks)
        if blk_max == 0:
            continue
        C = max(128, -(-blk_max // 128) * 128)
        nch = -(-C // 512)
        while C % nch:
            C += 128
            nch = -(-C // 512)
        cw = C // nch

        key = (C, nch, cw)
        if key not in _cache:
            _cache[key] = _build(C, nch, cw)
        nc = _cache[key]

        in_maps = [
            _pack_inputs(e, xf_b16, blk_toks[e], counts, C, w) for e in range(E)
        ]
        res = run_bass_kernel_spmd(nc, in_maps, core_ids=list(range(8)))
        _last_results = res

        for e in range(E):
            n = len(blk_toks[e])
            if n == 0:
                continue
            y = res.results[e]["y"].reshape(H, C)  # [H, C]; row h = k*128+p
            wgt = wts_all[e][blk * CB: blk * CB + n]
            out_f[blk_toks[e]] += wgt[:, None] * y[:, :n].T

    return out_f.reshape(b, s, H), aux_loss


# revision 5
# speedup vs baseline: 1.3052x; 1.3052x over previous
"""MoE FFN (flylora + ERA) Trainium2 kernel.

Expert-parallel over 8 NeuronCores: core e holds expert e's weights and
processes the tokens routed to it (top-2 routing computed on host as part of
the sharding step). All heavy math — the three [*,1024]x[1024,2048]-class
matmuls per expert plus the LoRA chain and ERA activation — runs on device.

Device-side layout trick: everything is computed transposed ([feature, token]),
so every matmul's stationary operand comes straight from the natural weight
layout and no on-device transposes are needed:
    hT[I,C]  = basW[H,I] as lhsT tiles, xeT[H,C] as moving     (mm1, both branches)
    loT[R,C] = downW[H,R] as lhsT, xeT moving                   (lora down)
    hT      += SCALING*upW[R,I] as lhsT, loT moving             (lora up, same psum)
    yT[H,C]  = downProj[I,H] as lhsT, prodT[I,C] moving         (mm2)
Biases fold to one per-partition vector: cb = base_b + S*(up_b + down_b@up_w).
ERA(x) = gelu_tanh(x) + 0.1*softplus(x); softplus = Ln(1+Exp(x)) because the
deployed ACT tables have no Softplus entry. Activation instructions are
batched per table set (Exp / Ln / Gelu_apprx_tanh live in three different
sets) so only 3 ACT table loads happen. Up-branch and output evacuations run
on the Vector engine to keep ScalarE off the PSUM-recycle critical path, and
one shared 4-slot PSUM pool lets the PE run several tiles ahead of the
evacuations.
"""

import sys

sys.path.insert(0, "/opt/trn_rl_repo")

import numpy as np
import ml_dtypes

import concourse.bass as bass  # noqa: F401
import concourse.mybir as mybir
import concourse.tile as tile
from concourse import bacc
from concourse.bass_utils import run_bass_kernel_spmd

H = 1024
I = 2048
E = 8
TOP_K = 2
R = 16
SCALING = 32.0 / 16.0
GAMMA = 0.1
KH = H // 128   # 8  k-tiles for H contraction
MI = I // 128   # 16 m-tiles over I
MH = H // 128   # 8  m-tiles over H (mm2 output)
KI = I // 128   # 16 k-tiles for I contraction
GRP = 4         # m-tiles per mm1 weight DMA slab
DGRP = 2        # m-tiles per mm2 weight DMA slab
QMI = 4         # m-tiles per batched activation quarter

AF = mybir.ActivationFunctionType
DT = mybir.dt

_cache: dict = {}
_last_results = None  # BassKernelResults of the most recent run (for profiling)


def _build(C: int, nch: int, cw: int):
    nc = bacc.Bacc("TRN2", target_bir_lowering=False, debug=False, num_devices=8)

    d_xe = nc.dram_tensor("xe", [128, KH, C], DT.bfloat16, kind="ExternalInput").ap()
    d_gw = nc.dram_tensor("gw", [MI // GRP, 128, GRP * H], DT.bfloat16, kind="ExternalInput").ap()
    d_uw = nc.dram_tensor("uw", [MI // GRP, 128, GRP * H], DT.bfloat16, kind="ExternalInput").ap()
    d_dw = nc.dram_tensor("dw", [MH // DGRP, 128, DGRP * I], DT.bfloat16, kind="ExternalInput").ap()
    d_gdw = nc.dram_tensor("gdw", [128, KH * R], DT.bfloat16, kind="ExternalInput").ap()
    d_udw = nc.dram_tensor("udw", [128, KH * R], DT.bfloat16, kind="ExternalInput").ap()
    d_guw = nc.dram_tensor("guw", [R, I], DT.bfloat16, kind="ExternalInput").ap()
    d_uuw = nc.dram_tensor("uuw", [R, I], DT.bfloat16, kind="ExternalInput").ap()
    d_cbg = nc.dram_tensor("cbg", [128, MI], DT.float32, kind="ExternalInput").ap()
    d_cbu = nc.dram_tensor("cbu", [128, MI], DT.float32, kind="ExternalInput").ap()
    d_dbb = nc.dram_tensor("dbb", [128, MH], DT.float32, kind="ExternalInput").ap()
    d_y = nc.dram_tensor("y", [MH, 128, C], DT.float32, kind="ExternalOutput").ap()

    from contextlib import ExitStack

    with tile.TileContext(nc) as tc, ExitStack() as ctx:
        consts = ctx.enter_context(tc.tile_pool(name="consts", bufs=1))
        big = ctx.enter_context(tc.tile_pool(name="big", bufs=1))
        wpool = ctx.enter_context(tc.tile_pool(name="wpool", bufs=2))
        dpool = ctx.enter_context(tc.tile_pool(name="dpool", bufs=2))
        ypool = ctx.enter_context(tc.tile_pool(name="ypool", bufs=3))
        psum_bufs = 4 if nch >= 2 else 8
        ps = ctx.enter_context(tc.tile_pool(name="ps", bufs=psum_bufs, space="PSUM"))

        # --- constant / input loads ---
        xe = consts.tile([128, KH, C], DT.bfloat16)
        nc.sync.dma_start(out=xe, in_=d_xe)
        gdw = consts.tile([128, KH * R], DT.bfloat16)
        nc.sync.dma_start(out=gdw, in_=d_gdw)
        udw = consts.tile([128, KH * R], DT.bfloat16)
        nc.sync.dma_start(out=udw, in_=d_udw)
        guw = consts.tile([R, I], DT.bfloat16)
        nc.sync.dma_start(out=guw, in_=d_guw)
        uuw = consts.tile([R, I], DT.bfloat16)
        nc.sync.dma_start(out=uuw, in_=d_uuw)
        cbg = consts.tile([128, MI], DT.float32)
        nc.sync.dma_start(out=cbg, in_=d_cbg)
        cbu = consts.tile([128, MI], DT.float32)
        nc.sync.dma_start(out=cbu, in_=d_cbu)
        dbb = consts.tile([128, MH], DT.float32)
        nc.sync.dma_start(out=dbb, in_=d_dbb)

        # --- lora down: loT[R, C] = down_w.T @ x.T, accumulated over KH ---
        lo_g = consts.tile([R, C], DT.bfloat16)
        lo_u = consts.tile([R, C], DT.bfloat16)
        for dwn, lo in ((gdw, lo_g), (udw, lo_u)):
            pl = ps.tile([R, nch, 512], DT.float32, tag="acc")
            for k in range(KH):
                for j in range(nch):
                    nc.tensor.matmul(
                        pl[:, j, :cw],
                        dwn[:, k * R:(k + 1) * R],
                        xe[:, k, j * cw:(j + 1) * cw],
                        start=(k == 0),
                        stop=(k == KH - 1),
                        skip_group_check=True,
                    )
            for j in range(nch):
                nc.vector.tensor_copy(lo[:, j * cw:(j + 1) * cw], pl[:, j, :cw])

        # --- big working buffers ---
        A = big.tile([128, MI, C], DT.float32)    # gate pre-act, then gelu(pre) in place
        Bb = big.tile([128, MI, C], DT.float32)   # exp(pre), then softplus in place
        U = big.tile([128, MI, C], DT.bfloat16)   # up branch (bias applied)
        GB = big.tile([128, MI, C], DT.bfloat16)  # ERA(gate)
        P = big.tile([128, MI, C], DT.bfloat16)   # gate*up, mm2 moving operand

        def mm1_tile(wslab, mi, upw, lo, m):
            pg = ps.tile([128, nch, 512], DT.float32, tag="acc")
            for k in range(KH):
                for j in range(nch):
                    nc.tensor.matmul(
                        pg[:, j, :cw],
                        wslab[:, mi * H + k * 128: mi * H + (k + 1) * 128],
                        xe[:, k, j * cw:(j + 1) * cw],
                        start=(k == 0),
                        stop=False,
                        skip_group_check=True,
                    )
            for j in range(nch):
                nc.tensor.matmul(
                    pg[:, j, :cw],
                    upw[:, m * 128:(m + 1) * 128],
                    lo[:, j * cw:(j + 1) * cw],
                    start=False,
                    stop=True,
                    skip_group_check=True,
                )
            return pg

        # gate branch: ACT evac (Identity+bias) to A; Exp quarters batched in
        for g in range(MI // GRP):
            wslab = wpool.tile([128, GRP * H], DT.bfloat16)
            nc.sync.dma_start(out=wslab, in_=d_gw[g])
            for mi in range(GRP):
                m = g * GRP + mi
                pg = mm1_tile(wslab, mi, guw, lo_g, m)
                nc.scalar.activation(
                    A[:, m, :].rearrange("p (j c) -> p j c", j=nch),
                    pg[:, :, :cw],
                    AF.Identity,
                    bias=cbg[:, m:m + 1],
                )
            if (g + 1) * GRP % QMI == 0:
                q0 = (g + 1) * GRP - QMI
                a_q = A[:, q0:q0 + QMI, :].rearrange("p m c -> p (m c)")
                b_q = Bb[:, q0:q0 + QMI, :].rearrange("p m c -> p (m c)")
                nc.scalar.activation(b_q, a_q, AF.Exp)

        # softplus: all Ln ops contiguous (one table switch)
        for q in range(MI // QMI):
            b_q = Bb[:, q * QMI:(q + 1) * QMI, :].rearrange("p m c -> p (m c)")
            nc.scalar.activation(b_q, b_q, AF.Ln, bias=1.0)

        # up branch: DVE evac (add bias, cast bf16) to U; per quarter finish
        # the gate path (Gelu on ACT, combine+product on DVE)
        for g in range(MI // GRP):
            wslab = wpool.tile([128, GRP * H], DT.bfloat16)
            nc.sync.dma_start(out=wslab, in_=d_uw[g])
            for mi in range(GRP):
                m = g * GRP + mi
                pg = mm1_tile(wslab, mi, uuw, lo_u, m)
                nc.vector.tensor_scalar_add(
                    U[:, m, :].rearrange("p (j c) -> p j c", j=nch),
                    pg[:, :, :cw],
                    cbu[:, m:m + 1],
                )
            if (g + 1) * GRP % QMI == 0:
                q0 = (g + 1) * GRP - QMI
                a_q = A[:, q0:q0 + QMI, :].rearrange("p m c -> p (m c)")
                b_q = Bb[:, q0:q0 + QMI, :].rearrange("p m c -> p (m c)")
                u_q = U[:, q0:q0 + QMI, :].rearrange("p m c -> p (m c)")
                gb_q = GB[:, q0:q0 + QMI, :].rearrange("p m c -> p (m c)")
                p_q = P[:, q0:q0 + QMI, :].rearrange("p m c -> p (m c)")
                nc.scalar.activation(a_q, a_q, AF.Gelu_apprx_tanh)
                nc.vector.scalar_tensor_tensor(
                    gb_q, b_q, GAMMA, a_q,
                    mybir.AluOpType.mult, mybir.AluOpType.add,
                )
                nc.vector.tensor_mul(p_q, gb_q, u_q)

        # --- mm2: yT[H, C] = down_w.T @ prodT, accumulated over KI ---
        for g in range(MH // DGRP):
            dslab = dpool.tile([128, DGRP * I], DT.bfloat16)
            nc.sync.dma_start(out=dslab, in_=d_dw[g])
            for mi in range(DGRP):
                m = g * DGRP + mi
                py = ps.tile([128, nch, 512], DT.float32, tag="acc")
                for k in range(KI):
                    for j in range(nch):
                        nc.tensor.matmul(
                            py[:, j, :cw],
                            dslab[:, mi * I + k * 128: mi * I + (k + 1) * 128],
                            P[:, k, j * cw:(j + 1) * cw],
                            start=(k == 0),
                            stop=(k == KI - 1),
                            skip_group_check=True,
                        )
                yt = ypool.tile([128, C], DT.float32)
                nc.vector.tensor_scalar_add(
                    yt.rearrange("p (j c) -> p j c", j=nch),
                    py[:, :, :cw],
                    dbb[:, m:m + 1],
                )
                nc.sync.dma_start(out=d_y[m], in_=yt)

    nc.compile()
    return nc


def _pack_inputs(e, xf_b16, toks, C, w):
    """Per-core input map for expert e; token block already chosen."""
    n = len(toks)
    xe = np.zeros((H, C), dtype=ml_dtypes.bfloat16)
    if n:
        xe[:, :n] = xf_b16[toks].T
    xe = np.ascontiguousarray(
        xe.reshape(KH, 128, C).transpose(1, 0, 2))  # [128, KH, C]

    def pack_mm1(wt):  # [H, I] -> [MI//GRP, 128, GRP*H]
        t = wt.reshape(KH, 128, MI, 128).transpose(2, 1, 0, 3).reshape(MI, 128, H)
        return np.ascontiguousarray(
            t.reshape(MI // GRP, GRP, 128, H).transpose(0, 2, 1, 3)
            .reshape(MI // GRP, 128, GRP * H))

    def pack_mm2(wt):  # [I, H] -> [MH//DGRP, 128, DGRP*I]
        t = wt.reshape(KI, 128, MH, 128).transpose(2, 1, 0, 3).reshape(MH, 128, I)
        return np.ascontiguousarray(
            t.reshape(MH // DGRP, DGRP, 128, I).transpose(0, 2, 1, 3)
            .reshape(MH // DGRP, 128, DGRP * I))

    b16 = ml_dtypes.bfloat16
    gw = pack_mm1(w["gate_base_w"][e].astype(b16))
    uw = pack_mm1(w["up_base_w"][e].astype(b16))
    dw = pack_mm2(w["down_w"][e].astype(b16))
    gdw = np.ascontiguousarray(
        w["gate_down_w"][e].astype(b16).reshape(KH, 128, R).transpose(1, 0, 2)
        .reshape(128, KH * R))
    udw = np.ascontiguousarray(
        w["up_down_w"][e].astype(b16).reshape(KH, 128, R).transpose(1, 0, 2)
        .reshape(128, KH * R))
    guw = (SCALING * w["gate_up_w"][e]).astype(b16)
    uuw = (SCALING * w["up_up_w"][e]).astype(b16)
    cbg = (w["gate_base_b"][e].astype(np.float64)
           + SCALING * (w["gate_up_b"][e].astype(np.float64)
                        + w["gate_down_b"][e].astype(np.float64)
                        @ w["gate_up_w"][e].astype(np.float64))).astype(np.float32)
    cbu = (w["up_base_b"][e].astype(np.float64)
           + SCALING * (w["up_up_b"][e].astype(np.float64)
                        + w["up_down_b"][e].astype(np.float64)
                        @ w["up_up_w"][e].astype(np.float64))).astype(np.float32)
    return {
        "xe": xe, "gw": gw, "uw": uw, "dw": dw, "gdw": gdw, "udw": udw,
        "guw": guw, "uuw": uuw,
        "cbg": np.ascontiguousarray(cbg.reshape(MI, 128).T),
        "cbu": np.ascontiguousarray(cbu.reshape(MI, 128).T),
        "dbb": np.ascontiguousarray(
            w["down_b"][e].astype(np.float32).reshape(MH, 128).T),
    }


def kernel(**inputs):
    global _last_results
    w = {k: np.asarray(v) for k, v in inputs.items()}
    x = w["x"]
    b, s, _ = x.shape
    T = b * s
    xf = x.reshape(T, H).astype(np.float32)

    # --- router (host; this determines the sharding) ---
    logits = xf @ w["router_w"].astype(np.float32) + w["router_b"].astype(np.float32)
    mx = logits.max(-1, keepdims=True)
    ex = np.exp(logits - mx)
    probs = ex / ex.sum(-1, keepdims=True)
    ti = np.argsort(-probs, axis=-1, kind="stable")[:, :TOP_K]
    tp = np.take_along_axis(probs, ti, axis=-1)
    tw = tp / tp.sum(-1, keepdims=True)

    p_mean = probs.mean(axis=0)
    f = np.bincount(ti.ravel(), minlength=E).astype(np.float32) / (T * TOP_K)
    aux_loss = np.float32(E * np.sum(f * p_mean))

    toks_all, wts_all = [], []
    for e in range(E):
        t_idx, slot = np.nonzero(ti == e)
        toks_all.append(t_idx)
        wts_all.append(tw[t_idx, slot].astype(np.float32))
    counts = np.array([len(t) for t in toks_all])

    xf_b16 = xf.astype(ml_dtypes.bfloat16)
    out_f = np.zeros((T, H), dtype=np.float32)

    # token blocks of at most 1024 per expert per kernel launch
    CB = 1024
    n_blocks = max(1, int(-(-counts.max() // CB)))
    for blk in range(n_blocks):
        blk_toks = [t[blk * CB:(blk + 1) * CB] for t in toks_all]
        blk_max = max(len(t) for t in blk_toks)
        if blk_max == 0:
            continue
        C = max(128, -(-blk_max // 128) * 128)
        nch = -(-C // 512)
        while C % nch:
            C += 128
            nch = -(-C // 512)
        cw = C // nch

        key = (C, nch, cw)
        if key not in _cache:
            _cache[key] = _build(C, nch, cw)
        nc = _cache[key]

        in_maps = [
            _pack_inputs(e, xf_b16, blk_toks[e], C, w) for e in range(E)
        ]
        res = run_bass_kernel_spmd(nc, in_maps, core_ids=list(range(8)))
        _last_results = res

        for e in range(E):
            n = len(blk_toks[e])
            if n == 0:
                continue
            y = res.results[e]["y"].reshape(H, C)  # [H, C]; row h = k*128+p
            wgt = wts_all[e][blk * CB: blk * CB + n]
            out_f[blk_toks[e]] += wgt[:, None] * y[:, :n].T

    return out_f.reshape(b, s, H), aux_loss
